# revision 9
# baseline (speedup 1.0000x reference)
"""Causal self-attention Trainium2 kernel.

Problem: y = CausalSelfAttention(x) with B=4, T=2048, C=1024, H=16 heads,
head_dim D=64, qkv split order (k, q, v), softmax scale C**-0.5.

Sharding (8 cores): core = 2*b + g  -> batch b in 0..3, head-group g in 0..1
(8 heads per group).  Each core computes, for its batch and its 8 heads:
  qkv partial matmuls, causal attention, and the partial output projection
  y_partial = att_out @ W_proj[rows of this head group].
The host sums the two partial projections per batch (row-parallel tensor
parallelism reduced on host during unsharding).

Device layout notes (per core):
  xT    [128, 8, 2048]  x^T (C on partitions), loaded via DMA transpose (bf16)
  kqT   [128, 8, 2048]  (x @ W_kq)^T : blocks 0-3 = k-channels, 4-7 = q-channels
                         head h: 64*(h%2) partition offset, block h//2 (+4 for q)
  v_aug [128, 16, 520]  v in natural layout, 65 cols/head = [v(64) | ones(1)]
  S^T   [k partitions, q free] per 128k x 512q block -> exp on ACT (scale 1/32)
        -> causal zeroing via gpsimd affine_select on diagonal-band blocks
  AV:   out^T[65, q] = [V|1]^T @ P^T accumulated over k tiles; row 64 = softmax
        denominator.  reciprocal (DVE) -> partition-broadcast (DMA) -> multiply.
  proj: y^T[1024, 2048] = W_proj_g^T(lhsT) @ att^T, streamed to HBM in fp32.
"""

import numpy as np
import ml_dtypes

B, T, C, H = 4, 2048, 1024, 16
D = C // H          # 64
HPC = H // 2        # 8 heads per core
CG = C // 2         # 512 channels per head group
P = 128

_compiled = {}


def _build(t=T):
    import concourse.bacc as bacc
    import concourse.tile as tile
    import concourse.mybir as mybir
    import concourse.bass as bass

    f32 = mybir.dt.float32
    bf16 = mybir.dt.bfloat16

    KT = C // P            # 8 contraction tiles over C
    MB = (2 * CG) // P     # 8 kq channel blocks (0-3 k, 4-7 q)
    TT = t // P            # token tiles of 128
    QC = t // 512          # q chunks of 512
    VB = CG // P           # 4 v/att channel blocks
    SCALE = float(C) ** -0.5

    nc = bacc.Bacc("TRN2", target_bir_lowering=False, debug=False,
                   num_devices=8)

    x_d = nc.dram_tensor("x", [t, C], bf16, kind="ExternalInput")
    wkq_d = nc.dram_tensor("wkq", [C, 2 * CG], bf16, kind="ExternalInput")
    wv_d = nc.dram_tensor("wv", [C, CG], bf16, kind="ExternalInput")
    wp_d = nc.dram_tensor("wp", [CG, C], bf16, kind="ExternalInput")
    y_d = nc.dram_tensor("y", [C, t], f32, kind="ExternalOutput")

    with tile.TileContext(nc) as tc:
        with (
            tc.tile_pool(name="persist", bufs=1) as persist,
            tc.tile_pool(name="psA", bufs=4, space="PSUM") as psA,
            tc.tile_pool(name="avP", bufs=1, space="PSUM") as avP,
            tc.tile_pool(name="ptP", bufs=36) as ptP,
            tc.tile_pool(name="rcP", bufs=2) as rcP,
            tc.tile_pool(name="rbP", bufs=2) as rbP,
            tc.tile_pool(name="atP", bufs=3) as atP,
            tc.tile_pool(name="yP", bufs=3) as yP,
        ):
            xT = persist.tile([P, KT, t], bf16)
            wkq_sb = persist.tile([P, KT, 2 * CG], bf16)
            wv_sb = persist.tile([P, KT, CG], bf16)
            wp_sb = persist.tile([P, VB, C], bf16)
            kqT = persist.tile([P, MB, t], bf16)
            v_aug = persist.tile([P, TT, HPC * (D + 1)], bf16)
            att = persist.tile([P, VB, t], bf16)

            # ---- loads ----
            for ct in range(KT):
                nc.sync.dma_start(xT[:, ct, :], x_d[:, ct * P:(ct + 1) * P],
                                  transpose=True)
            nc.sync.dma_start(
                wkq_sb, wkq_d.ap().rearrange("(kt p) m -> p kt m", p=P))
            nc.sync.dma_start(
                wv_sb, wv_d.ap().rearrange("(kt p) m -> p kt m", p=P))
            nc.sync.dma_start(
                wp_sb, wp_d.ap().rearrange("(kt p) m -> p kt m", p=P))
            nc.vector.memset(v_aug, 1.0)

            def emit_kq_block(mb):
                for c in range(QC):
                    ps = psA.tile([P, 512], f32, bufs=2)
                    for kt in range(KT):
                        nc.tensor.matmul(
                            ps,
                            lhsT=wkq_sb[:, kt, mb * P:(mb + 1) * P],
                            rhs=xT[:, kt, c * 512:(c + 1) * 512],
                            start=(kt == 0), stop=(kt == KT - 1))
                    nc.vector.tensor_copy(kqT[:, mb, c * 512:(c + 1) * 512],
                                          ps)

            def emit_v():
                for tt in range(TT):
                    ps = psA.tile([P, CG], f32, bufs=2)
                    for kt in range(KT):
                        nc.tensor.matmul(
                            ps,
                            lhsT=xT[:, kt, tt * P:(tt + 1) * P],
                            rhs=wv_sb[:, kt, :],
                            start=(kt == 0), stop=(kt == KT - 1))
                    nc.vector.tensor_copy(
                        v_aug[:, tt, :].rearrange("p (h e) -> p h e",
                                                  e=D + 1)[:, :, 0:D],
                        ps[:].rearrange("p (h d) -> p h d", d=D))

            def emit_attn(hp):
                for c in range(QC):
                    J = 4 * (c + 1)
                    avp = [avP.tile([D + 1, 512], f32, name=f"avp{hi}",
                                    tag=f"avp{hi}")
                           for hi in range(2)]
                    pts = []
                    for j in range(J):
                        row = []
                        for hi in range(2):
                            lo = D * hi
                            st = psA.tile([P, 512], f32)
                            nc.tensor.matmul(
                                st,
                                lhsT=kqT[lo:lo + D, hp, j * P:(j + 1) * P],
                                rhs=kqT[lo:lo + D, 4 + hp,
                                        c * 512:(c + 1) * 512],
                                start=True, stop=True)
                            pt = ptP.tile([P, 512], bf16)
                            nc.scalar.activation(
                                pt, st, mybir.ActivationFunctionType.Exp,
                                scale=SCALE)
                            if j >= 4 * c:
                                nc.gpsimd.affine_select(
                                    pt, pt,
                                    pattern=[[1, 512]],
                                    compare_op=mybir.AluOpType.is_ge,
                                    fill=0.0,
                                    base=512 * c - P * j,
                                    channel_multiplier=-1)
                            row.append(pt)
                        pts.append(row)
                    for hi in range(2):
                        h = 2 * hp + hi
                        for j in range(J):
                            nc.tensor.matmul(
                                avp[hi],
                                lhsT=v_aug[:, j,
                                           h * (D + 1):(h + 1) * (D + 1)],
                                rhs=pts[j][hi],
                                start=(j == 0), stop=(j == J - 1),
                                skip_group_check=True)
                        rc = rcP.tile([D + 1, 512], f32)
                        nc.vector.reciprocal(rc[D:D + 1, :],
                                             avp[hi][D:D + 1, :])
                        # partition_broadcast only reads physical partition 0;
                        # DMA-shift the reciprocal row there first.
                        rc0 = rcP.tile([1, 512], f32, name="rc0", tag="rc0")
                        nc.sync.dma_start(rc0, rc[D:D + 1, :])
                        rb = rbP.tile([D, 512], f32)
                        nc.gpsimd.partition_broadcast(rb, rc0[0:1, :],
                                                      channels=D)
                        at = atP.tile([D, 512], bf16)
                        nc.vector.tensor_mul(at, avp[hi][0:D, :], rb)
                        nc.sync.dma_start(
                            att[D * hi:D * (hi + 1), hp,
                                c * 512:(c + 1) * 512],
                            at)

            # qkv for head-pair hp needs kq blocks hp (k) and 4+hp (q)
            emit_kq_block(0)
            emit_kq_block(4)
            emit_v()
            emit_attn(0)
            for hp in range(1, 4):
                emit_kq_block(hp)
                emit_kq_block(4 + hp)
                emit_attn(hp)

            # ---- projection: y^T = W_proj_g^T @ att^T ----
            for mb in range(C // P):
                for c in range(QC):
                    ps = psA.tile([P, 512], f32, bufs=2)
                    for kt in range(VB):
                        nc.tensor.matmul(
                            ps,
                            lhsT=wp_sb[:, kt, mb * P:(mb + 1) * P],
                            rhs=att[:, kt, c * 512:(c + 1) * 512],
                            start=(kt == 0), stop=(kt == VB - 1))
                    yt = yP.tile([P, 512], f32)
                    nc.vector.tensor_copy(yt, ps)
                    nc.sync.dma_start(
                        y_d[mb * P:(mb + 1) * P, c * 512:(c + 1) * 512], yt)

    nc.compile()
    return nc


def _get_compiled(t=T):
    if t not in _compiled:
        _compiled[t] = _build(t)
    return _compiled[t]


def make_in_maps(x, W_qkv, W_proj):
    bf = ml_dtypes.bfloat16
    in_maps = []
    for core in range(8):
        b, g = core // 2, core % 2
        in_maps.append({
            "x": np.ascontiguousarray(x[b]).astype(bf),
            "wkq": np.concatenate(
                [W_qkv[:, g * CG:(g + 1) * CG],
                 W_qkv[:, C + g * CG:C + (g + 1) * CG]], axis=1).astype(bf),
            "wv": np.ascontiguousarray(
                W_qkv[:, 2 * C + g * CG:2 * C + (g + 1) * CG]).astype(bf),
            "wp": np.ascontiguousarray(
                W_proj[g * CG:(g + 1) * CG, :]).astype(bf),
        })
    return in_maps


def kernel(x, W_qkv, W_proj, _trace=False):
    import concourse.bass_utils as bass_utils

    nc = _get_compiled()
    in_maps = make_in_maps(x, W_qkv, W_proj)
    res = bass_utils.run_bass_kernel_spmd(
        nc, in_maps, core_ids=list(range(8)), trace=_trace)
    y = np.zeros((B, T, C), np.float32)
    for core in range(8):
        y[core // 2] += res.results[core]["y"].T
    if _trace:
        kernel.last_results = res
    return y
